# revision 2
# baseline (speedup 1.0000x reference)
"""Trainium2 Bass kernel for nn_ExpandFormerV15Complete (moe_routing) — v3.

Design:
  * Host-side routing/sharding: tokens are classed by (domain | none) and
    dealt evenly across the 8 cores, so every core runs the identical
    static program (it depends only on the 9 class sizes); the per-core
    token->slot assignment is data. The device does all the math of the
    reference path (embed gather, W1 matmul, exact GELU, W2 matmul,
    h + 0.1*corr) — but the expert MLP runs only for domain-member
    tokens, and only for their one domain.
  * Vocab compaction: member vocab rows (~25k) and non-member vocab rows
    (~25k) are each compacted into their own table half, so every gather
    index fits the gather's int16 index type with a single table.
  * Gather: dma_gather of 256B f32 embed rows (exact h), chunks spread
    across 4 SWDGE queues (num_swdge_queues=4) whose descriptor rings
    drain in parallel (~2.6ns/idx aggregate vs ~9ns/idx on one queue).
  * The gather ucode library load (~10us stream) is hoisted to the very
    first instruction (before the Tile context) so it overlaps the
    framework preamble and input DMAs.
  * Non-member slots need y = h only: their gather output is DMAed
    straight back out with no engine work, token-major. Member slots are
    transposed (PE) to feature-major once per gather chunk, run through
    the segment-aligned MLP (one matmul pair per domain), and written
    out feature-major; the host merges both layouts.
"""

import numpy as np
import ml_dtypes

import concourse.bass as bass
import concourse.bacc as bacc
import concourse.tile as tile
import concourse.mybir as mybir
from concourse import library_config
from concourse.bass_utils import run_bass_kernel_spmd

VOCAB = 50257
BASE = 64
NDOM = 8
HID = 128
B, S = 16, 2048
CORR = 0.1

NCORES = 8
NTOK = B * S
HALF = 32768                    # none-table base row in the packed table
MCHUNK = 256                    # member gather chunk (fine for pipelining)
NCHUNK = 256                    # none gather chunk
NQ = 4                          # SWDGE queues

F32 = mybir.dt.float32
BF16 = mybir.dt.bfloat16
I16 = mybir.dt.int16


def _install_tile_fix():
    """This walrus build rejects Drain instructions with >1 sync wait.
    Tile's exit barrier attaches one wait per DMA-sem lane to its tail
    drain; split them into a chain of single-wait drains."""
    if getattr(tile.TileContext, "_drain_split_installed", False):
        return

    def _patched(self, tick_clock, wait_clock):
        from concourse.vector_clock import ScopedClock

        drain_inst = self.nc.sync.drain()
        wait_clock.add_sem_waits(
            drain_inst.ins, ScopedClock({None: tick_clock.global_clock})
        )
        si = drain_inst.ins.sync_info
        if si is not None and si.on_wait and len(si.on_wait) > 1:
            waits = list(si.on_wait)
            si.on_wait = waits[:1]
            for w in waits[1:]:
                d2 = self.nc.sync.drain()
                si2 = d2.ins.sync_info
                if si2 is None:
                    d2.ins.sync_info = type(si)(on_wait=[w], on_update=[])
                else:
                    si2.on_wait = list(si2.on_wait) + [w]
        self.nc.all_engine_barrier()
        popped = self.nc._tile_sem_poison_stack.pop()
        assert popped is self._sem_poison
        self.nc.clear_and_free_semaphores(list(self.sems.allocated().values()))
        self.nc.all_engine_barrier()

    tile.TileContext._drain_and_barrier = _patched
    tile.TileContext._drain_split_installed = True


def _r128(n):
    return (n + 127) & ~127


def _make_plan(x, member):
    xf = np.asarray(x).reshape(-1).astype(np.int64)
    mem = np.asarray(member, dtype=np.float32)

    vdom = np.where(mem.sum(1) > 0, mem.argmax(1), -1)      # [VOCAB]
    vrows_m = np.nonzero(vdom >= 0)[0]
    vrows_n = np.nonzero(vdom < 0)[0]
    assert len(vrows_m) < HALF and len(vrows_n) < HALF
    vmap = np.zeros(VOCAB, np.int64)
    vmap[vrows_m] = np.arange(len(vrows_m))
    vmap[vrows_n] = np.arange(len(vrows_n))

    dom = vdom[xf]                                           # [N] per token
    cls = np.where(dom >= 0, dom, NDOM)                      # 0..7 member, 8 none
    order = np.argsort(cls, kind="stable")
    cnt = np.bincount(cls, minlength=NDOM + 1).astype(np.int64)
    seg = [int(-(-c // NCORES)) for c in cnt]

    M = sum(seg[:NDOM])
    Mp = _r128(M)
    Np = _r128(seg[NDOM])
    TOT = Mp + Np

    offs = []
    cur = 0
    for k in range(NDOM):
        offs.append(cur); cur += seg[k]
    offs.append(Mp)

    slot_tok = np.full((NCORES, TOT), -1, np.int64)
    idx16 = np.zeros((NCORES, TOT), np.int16)
    pos = 0
    for k in range(NDOM + 1):
        n = int(cnt[k])
        toks = order[pos:pos + n]; pos += n
        cores = np.arange(n) % NCORES
        sl = offs[k] + np.arange(n) // NCORES
        slot_tok[cores, sl] = toks
        base = 0 if k < NDOM else HALF
        idx16[cores, sl] = (vmap[xf[toks]] + 0).astype(np.int16)
        # none tokens index the second table half via the in_ap base; the
        # compact index itself is < 32768 either way.

    # gather chunks: (slot_off, nidx, tb) with tb 0=member half, 1=none half
    chunks = []
    p = 0
    while p < Mp:
        n = min(MCHUNK, Mp - p)
        chunks.append((p, n, 0))
        p += n
    while p < TOT:
        n = min(NCHUNK, TOT - p)
        chunks.append((p, n, 1))
        p += n
    # greedy queue assignment by descriptor load
    qload = [0] * NQ
    qassign = []
    for off, n, tb in chunks:
        q = min(range(NQ), key=lambda i: qload[i])
        qload[q] += n
        qassign.append(q)

    segments = [(offs[k], offs[k] + seg[k], k) for k in range(NDOM) if seg[k] > 0]

    key = tuple(seg)
    return dict(seg=seg, TOT=TOT, Mp=Mp, chunks=chunks, qassign=qassign,
                segments=segments, slot_tok=slot_tok, idx16=idx16,
                vrows_m=vrows_m, vrows_n=vrows_n, key=key)


def _build_program(TOT, Mp, chunks, qassign, segments):
    _install_tile_fix()
    nc = bacc.Bacc("TRN2", target_bir_lowering=False, debug=False,
                   num_swdge_queues=NQ)

    JTOT = TOT // 128
    MJ = Mp // 128
    xpw_in = nc.declare_dram_parameter("xpw", [128, TOT // 16], I16, isOutput=False)
    t_in = nc.declare_dram_parameter("t", [2 * HALF, BASE], F32, isOutput=False)
    wid_in = nc.declare_dram_parameter("wid", [128, NDOM * (HID + BASE) + 128], BF16,
                                       isOutput=False)
    y_out = nc.declare_dram_parameter("y", [128, (JTOT - MJ) * BASE], F32, isOutput=True)
    y2_out = nc.declare_dram_parameter("y2", [BASE, Mp], F32, isOutput=True)

    with tile.TileContext(nc) as tc:
        # Start the gather-ucode library stream first thing: the ~10us ucode
        # stream overlaps the input DMAs (native instructions are not blocked
        # by an in-flight stream; only the first gather waits for it). It must
        # sit inside the Tile context — the context-entry barrier's engine
        # drain would otherwise stall every engine on the stream.
        nc.gpsimd.load_library(library_config.mlp)
        with (
            tc.tile_pool(name="const", bufs=1) as cpool,
            tc.tile_pool(name="h16", bufs=3) as hpool,
            tc.tile_pool(name="gelu", bufs=2) as gpool,
            tc.tile_pool(name="ps_t", bufs=2, space="PSUM") as ps_t,
            tc.tile_pool(name="ps_a", bufs=2, space="PSUM") as ps_a,
            tc.tile_pool(name="ps_c", bufs=2, space="PSUM") as ps_c,
        ):
            xpw = cpool.tile([128, TOT // 16], I16)
            nc.sync.dma_start(out=xpw[:, :], in_=xpw_in[:, :])
            wid = cpool.tile([128, NDOM * (HID + BASE) + 128], BF16)
            nc.sync.dma_start(out=wid[:, :], in_=wid_in[:, :])
            w = wid[:, 0:NDOM * (HID + BASE)]
            idn = wid[:, NDOM * (HID + BASE):]

            hbuf = cpool.tile([128, JTOT * BASE], F32, tag="hbuf")
            hT = cpool.tile([BASE, Mp], BF16, tag="hT")
            y2b = cpool.tile([BASE, Mp], F32, tag="y2b")

            def emit_segment(a, b, d):
                """Expert MLP for one domain segment of member slots,
                feature-major throughout; y2 slice streams out at the end."""
                wseg = b - a
                psa = ps_a.tile([128, 512], F32)
                nc.tensor.matmul(
                    psa[:, 0:wseg],
                    lhsT=w[0:BASE, d * HID:(d + 1) * HID],
                    rhs=hT[:, a:b],
                    start=True, stop=True,
                )
                G = gpool.tile([128, 512], BF16, tag="G")
                nc.scalar.activation(G[:, 0:wseg], psa[:, 0:wseg],
                                     mybir.ActivationFunctionType.Gelu)
                psc = ps_c.tile([BASE, 512], F32)
                nc.tensor.matmul(
                    psc[:, 0:wseg],
                    lhsT=w[:, NDOM * HID + d * BASE:NDOM * HID + (d + 1) * BASE],
                    rhs=G[:, 0:wseg],
                    start=True, stop=True,
                )
                nc.vector.tensor_tensor(
                    out=y2b[:, a:b], in0=hT[:, a:b], in1=psc[:, 0:wseg],
                    op=mybir.AluOpType.add,
                )

            segq = list(segments)
            flush = [0, 0]          # segments emitted, y2 columns flushed

            def emit_seg_and_flush(a, b, d):
                emit_segment(a, b, d)
                flush[0] += 1
                # stream member output out every 2 segments (Scalar HWDGE
                # queue, so the Sync queue stays free for none-chunk writes)
                if flush[0] % 2 == 0 or flush[0] == len(segments):
                    nc.scalar.dma_start(out=y2_out[:, flush[1]:b],
                                        in_=y2b[:, flush[1]:b])
                    flush[1] = b
            for ci, (off, nidx, tb) in enumerate(chunks):
                src = t_in[tb * HALF:tb * HALF + HALF, :]
                sl = slice((off // 128) * BASE, ((off + nidx) // 128) * BASE)
                nc.gpsimd.dma_gather(
                    out_ap=hbuf[:, sl].rearrange("p (j e) -> p j e", e=BASE),
                    in_ap=src,
                    idxs_ap=xpw[:, off // 16:(off + nidx) // 16],
                    num_idxs=nidx, num_idxs_reg=nidx, elem_size=BASE,
                    single_packet=False, queue_num=qassign[ci],
                )
                if tb == 1:
                    # none-tokens: y = h, stream each chunk straight out as it
                    # lands (no barrier on the whole region)
                    nc.sync.dma_start(
                        out=y_out[:, ((off - Mp) // 128) * BASE:
                                  ((off - Mp + nidx) // 128) * BASE],
                        in_=hbuf[:, sl],
                    )
                    continue
                # member chunk: cast to bf16 and transpose to feature-major
                jn = nidx // 128
                hb16 = hpool.tile([128, (MCHUNK // 128) * BASE], BF16, tag="hb16")
                nc.vector.tensor_copy(out=hb16[:, 0:jn * BASE], in_=hbuf[:, sl])
                pst = ps_t.tile([BASE, MCHUNK], BF16)
                for jj in range(jn):
                    nc.tensor.matmul(
                        pst[:, jj * 128:(jj + 1) * 128],
                        lhsT=hb16[:, jj * BASE:(jj + 1) * BASE],
                        rhs=idn,
                        is_transpose=True, start=True, stop=True,
                    )
                nc.vector.tensor_copy(out=hT[:, off:off + nidx],
                                      in_=pst[:, 0:nidx])
                # emit any domain segment fully covered by the chunks so far,
                # so its MLP pipelines behind later chunks' gathers
                while segq and segq[0][1] <= off + nidx:
                    emit_seg_and_flush(*segq.pop(0))
            while segq:
                emit_seg_and_flush(*segq.pop(0))

    nc.compile()
    return nc


_CACHE = {}
_LAST = {}


def prepare_in_maps(x, embed, W1, W2, member):
    plan = _make_plan(x, member)

    embed = np.asarray(embed, dtype=np.float32)
    W1 = np.asarray(W1, dtype=np.float32)
    W2 = np.asarray(W2, dtype=np.float32)

    t = np.zeros((2 * HALF, BASE), np.float32)
    t[:len(plan["vrows_m"])] = embed[plan["vrows_m"]]
    t[HALF:HALF + len(plan["vrows_n"])] = embed[plan["vrows_n"]]

    wid = np.zeros((128, NDOM * (HID + BASE) + 128), np.float32)
    for d in range(NDOM):
        wid[:BASE, d * HID:(d + 1) * HID] = W1[d]
        wid[:, NDOM * HID + d * BASE:NDOM * HID + (d + 1) * BASE] = W2[d] * CORR
    wid[:, NDOM * (HID + BASE):] = np.eye(128, dtype=np.float32)
    wid = wid.astype(ml_dtypes.bfloat16)

    in_maps = []
    for c in range(NCORES):
        idx = plan["idx16"][c]
        xpw = np.tile(idx.reshape(-1, 16).T, (8, 1)).astype(np.int16)
        in_maps.append({"xpw": xpw, "t": t, "wid": wid})
    return in_maps, plan


def kernel(x, embed, W1, W2, member):
    in_maps, plan = prepare_in_maps(x, embed, W1, W2, member)

    nc = _CACHE.get(plan["key"])
    if nc is None:
        nc = _build_program(plan["TOT"], plan["Mp"], plan["chunks"],
                            plan["qassign"], plan["segments"])
        _CACHE[plan["key"]] = nc
    _LAST["nc"] = nc

    res = run_bass_kernel_spmd(nc, in_maps, core_ids=list(range(NCORES)))

    TOT, Mp = plan["TOT"], plan["Mp"]
    out = np.empty((NTOK, BASE), np.float32)
    for c in range(NCORES):
        st = plan["slot_tok"][c]
        # member slots: feature-major y2
        y2 = np.asarray(res.results[c]["y2"], dtype=np.float32)    # [64, Mp]
        mv = st[:Mp] >= 0
        out[st[:Mp][mv]] = y2[:, mv].T
        # none slots: token-major y
        y = np.asarray(res.results[c]["y"], dtype=np.float32)
        ys = y.reshape(128, (TOT - Mp) // 128, BASE).transpose(1, 0, 2).reshape(-1, BASE)
        nv = st[Mp:] >= 0
        out[st[Mp:][nv]] = ys[nv]
    return out.reshape(B, S, BASE)
